# revision 1
# baseline (speedup 1.0000x reference)
"""Linear-chain CRF loss (forward partition + gold score) on 8 Trainium2 cores.

Strategy: data-parallel over batch (8 sequences per core). The forward
algorithm runs in exp-space: state p_t[j, b] = exp(alpha_t - running log scale).
Per step: one 32x32x8 matmul on PE (sum_i expT[i,j] * p[i,b]) and one
elementwise multiply by exp(emission) on DVE. Every R=8 steps the state is
renormalized by its column sum (computed with a ones-matmul on PE) to keep
fp32 in range; the scales are logged so the host can reconstruct alpha.
The full state history [32, 512*8] is written back to HBM; the host picks the
column at t = len_b - 1 per sequence, adds back the log scales and the
T[:, END] column, and does the final 32-wide logsumexp + sums. The labeled
(gold path) score is pure tiny table gathers, done on host.
"""

import numpy as np

START_IDX = 29
END_IDX = 30
PAD_IDX = 31

B, S, L = 64, 512, 32
NCORES = 8
BPC = B // NCORES  # sequences per core
R = 8              # rescale period (worst-case growth ~e^9/step; 8 steps < e^88)
NR = S // R        # number of rescales

_nc = None


def _build_nc():
    import concourse.bacc as bacc
    import concourse.bass as bass
    import concourse.mybir as mybir
    from concourse import tile

    dt = mybir.dt.float32
    nc = bacc.Bacc(None, target_bir_lowering=False)

    em_in = nc.declare_dram_parameter("em", (L, S * BPC), dt, isOutput=False)
    tr_in = nc.declare_dram_parameter("tr", (L, L), dt, isOutput=False)
    ts_in = nc.declare_dram_parameter("tstart", (L, 1), dt, isOutput=False)
    p_out = nc.declare_dram_parameter("p_all", (L, S * BPC), dt, isOutput=True)
    s_out = nc.declare_dram_parameter("s_all", (1, NR * BPC), dt, isOutput=True)

    Exp = mybir.ActivationFunctionType.Exp

    with tile.TileContext(nc) as tc:
        with (
            tc.tile_pool(name="big", bufs=1) as big,
            tc.tile_pool(name="small", bufs=1) as small,
            tc.tile_pool(name="qp", bufs=3, space=bass.MemorySpace.PSUM) as qp,
            tc.tile_pool(name="sp", bufs=2, space=bass.MemorySpace.PSUM) as sp,
            tc.tile_pool(name="sc", bufs=2) as sc,
        ):
            E = big.tile([L, S * BPC], dt)    # exp(emissions), [j, t*8+b]
            P = big.tile([L, S * BPC], dt)    # state history,  [j, t*8+b]
            s_sb = big.tile([1, NR * BPC], dt)
            expT = small.tile([L, L], dt)     # exp(transition), [i, j]
            est = small.tile([L, 1], dt)      # exp(T[START, j]) as column
            ones_col = small.tile([L, 1], dt)
            ones_row = small.tile([1, L], dt)

            nc.sync.dma_start(E[:], em_in[:])
            nc.sync.dma_start(expT[:], tr_in[:])
            nc.sync.dma_start(est[:], ts_in[:])
            nc.gpsimd.memset(ones_col[:], 1.0)
            nc.gpsimd.memset(ones_row[:], 1.0)

            nc.scalar.activation(E[:], E[:], Exp)
            nc.scalar.activation(expT[:], expT[:], Exp)
            nc.scalar.activation(est[:], est[:], Exp)

            # p_0[j, b] = exp(T[START, j]) * E_0[j, b]
            nc.vector.tensor_scalar_mul(P[:, 0:BPC], E[:, 0:BPC], est[:, 0:1])

            for t in range(1, S):
                cur = P[:, t * BPC:(t + 1) * BPC]
                q = qp.tile([L, BPC], dt, tag="q")
                nc.tensor.matmul(
                    q[:], expT[:], P[:, (t - 1) * BPC:t * BPC],
                    start=True, stop=True,
                )
                nc.vector.tensor_mul(cur, q[:], E[:, t * BPC:(t + 1) * BPC])
                if t % R == R - 1:
                    r = t // R
                    s_ps = sp.tile([1, BPC], dt, tag="s")
                    nc.tensor.matmul(s_ps[:], ones_col[:], cur, start=True, stop=True)
                    nc.scalar.copy(s_sb[:, r * BPC:(r + 1) * BPC], s_ps[:])
                    rec = sc.tile([1, BPC], dt, tag="rec")
                    nc.vector.reciprocal(rec[:], s_ps[:])
                    bc = sp.tile([L, BPC], dt, tag="bc")
                    nc.tensor.matmul(bc[:], ones_row[:], rec[:], start=True, stop=True)
                    nc.vector.tensor_mul(cur, cur, bc[:])

            nc.sync.dma_start(p_out[:], P[:])
            nc.sync.dma_start(s_out[:], s_sb[:])

    nc.compile()
    return nc


def _labeled_score(lstm_scores, word_seq_lens, tags, mask, transition):
    b_idx = np.arange(B)
    t0 = tags[:, 0]
    begin = transition[START_IDX, t0].astype(np.float64) + lstm_scores[b_idx, 0, t0]
    prev, curt = tags[:, :-1], tags[:, 1:]
    trans_mid = transition[prev, curt].astype(np.float64)
    em_mid = np.take_along_axis(lstm_scores[:, 1:, :], curt[..., None], axis=2)[..., 0]
    mid = np.where(mask[:, 1:], trans_mid + em_mid, 0.0)
    end_ids = tags[b_idx, word_seq_lens - 1]
    end_sc = transition[end_ids, END_IDX].astype(np.float64)
    return begin.sum() + end_sc.sum() + mid.sum()


def kernel(lstm_scores, word_seq_lens, tags, mask, transition):
    global _nc
    lstm_scores = np.asarray(lstm_scores, dtype=np.float32)
    word_seq_lens = np.asarray(word_seq_lens).astype(np.int64)
    tags = np.asarray(tags).astype(np.int64)
    mask = np.asarray(mask).astype(bool)
    transition = np.asarray(transition, dtype=np.float32)

    if _nc is None:
        _nc = _build_nc()

    tstart = np.ascontiguousarray(transition[START_IDX, :].reshape(L, 1))
    in_maps = []
    for c in range(NCORES):
        em = lstm_scores[c * BPC:(c + 1) * BPC]               # (8, 512, 32)
        emT = np.ascontiguousarray(em.transpose(2, 1, 0)).reshape(L, S * BPC)
        in_maps.append({"em": emT, "tr": transition, "tstart": tstart})

    from concourse.bass_utils import run_bass_kernel_spmd
    res = run_bass_kernel_spmd(_nc, in_maps, list(range(NCORES)))

    t_end = transition[:, END_IDX].astype(np.float64)         # T[j, END]
    unlabeled = 0.0
    for c in range(NCORES):
        p_all = res.results[c]["p_all"].reshape(L, S, BPC)    # [j, t, b]
        s_all = res.results[c]["s_all"].reshape(1, NR, BPC)[0]  # [r, b]
        with np.errstate(divide="ignore"):
            logs = np.log(s_all.astype(np.float64))
            for b in range(BPC):
                t_star = int(word_seq_lens[c * BPC + b]) - 1
                col = p_all[:, t_star, b].astype(np.float64)
                nr = (t_star + 1) // R
                la = np.log(col) + logs[:nr, b].sum() + t_end
                m = la.max()
                unlabeled += m + np.log(np.exp(la - m).sum())

    labeled = _labeled_score(lstm_scores, word_seq_lens, tags, mask, transition)
    return (np.float32(unlabeled), np.float32(labeled))

